# revision 64
# baseline (speedup 1.0000x reference)
"""Trainium2 Bass kernel for windowed attention with relative-position bias.

Problem (hardcoded): x [32, 256, 25, 25] f32, w_qkv [256, 768], rel_emb [2401, 8],
w_out [256, 256], rel_idx [625, 625] int32. 8 heads of dim 32, n = 625 tokens.

Sharding: data-parallel over batch; 4 batches per core on 8 NeuronCores; weights
and bias replicated. No collectives.

Per-core dataflow (bf16 matmuls, f32 PSUM accumulate):
  qkv^T = w_qkv^T @ x          -> qT,kT tiles [32h..., 625]  (q pre-scaled on host)
  v     = x^T @ w_v            -> v tiles [125, 5jt, 8h, 1|dh|0] (ones col 0)
  sim^T = k_h^T q_h (+ bias^T via identity-matmul accumulate, raw bias bf16)
          pair-combined PSUM tiles [125, 2, 625]; paired heads on distinct PE
          row groups (2-way row tiling)
  es    = exp(sim^T)           (one ScalarE activation per pair-jt, -> bf16)
  av^T  = [1|v|0]^T @ es       col strips 0/64 in one shared PSUM tile
                                (partition-disjoint concurrent accum groups);
                                ones row 0/64 gives the softmax denominator
  outT  = av * bcast(1/denom)  (reciprocal_approx_fast at partition 0 + gpsimd
                                broadcast + per-head TT)
  out^T = w_outP^T @ outT      per-pair K blocks of host-permuted w_out -> HBM
"""

import sys

if "/opt/trn_rl_repo" not in sys.path:
    sys.path.insert(0, "/opt/trn_rl_repo")

import numpy as np
import ml_dtypes

B, D, WS = 32, 256, 25
N = WS * WS            # 625
H, DH = 8, 32
NC = 8                 # cores
BL = B // NC           # 4 batches per core
SCALE = DH ** -0.5
JT = 5                 # j tiles of 125
JP = N // JT           # 125
CHUNKS = ((0, 512), (512, 113))             # i chunks for 625-wide psum at offset 0
CHUNKS_M = (((0, 512), (512, 113)),         # bank-aligned chunks for sim[:, m, :]
            ((0, 399), (399, 226)))         # (m=1 starts at f32 offset 625)

_cache = {}


def _build():
    import concourse.bass as bass
    from concourse import bacc, mybir
    from concourse.tile import TileContext
    from concourse.masks import make_identity

    f32 = mybir.dt.float32
    bf16 = mybir.dt.bfloat16
    f8 = mybir.dt.float8e4

    nc = bacc.Bacc()
    x_ext = nc.declare_dram_parameter("x", [BL, D, N], bf16, isOutput=False)
    wqkv_ext = nc.declare_dram_parameter("wqkv", [D, 3 * D], bf16, isOutput=False)
    woutp_ext = nc.declare_dram_parameter("woutp", [128, 4, D], bf16, isOutput=False)
    biasT_ext = nc.declare_dram_parameter("biasT", [JP, H, JT * N], f8, isOutput=False)
    out_ext = nc.declare_dram_parameter("out", [BL, D, N], bf16, isOutput=True)

    with TileContext(nc) as tc:
        with (
            tc.tile_pool(name="const", bufs=1) as const,
            tc.tile_pool(name="xp", bufs=2) as xp,
            tc.tile_pool(name="qk", bufs=2) as qkp,
            tc.tile_pool(name="vp", bufs=2) as vp,
            tc.tile_pool(name="es", bufs=4) as esp,
            tc.tile_pool(name="rcp", bufs=3) as rcpp,
            tc.tile_pool(name="rb", bufs=2) as rbp,
            tc.tile_pool(name="ot", bufs=2) as otp,
            tc.tile_pool(name="res", bufs=2) as resp,
            tc.tile_pool(name="sim", bufs=3, space="PSUM") as simp,
            tc.tile_pool(name="avp", bufs=1, space="PSUM") as avp,
        ):
            wqkv_sb = const.tile([128, 2, 3 * D], bf16)
            nc.sync.dma_start(out=wqkv_sb, in_=wqkv_ext.rearrange("(k p) c -> p k c", p=128))
            # grouped head tiles, flat per-partition so each DMA is one big
            # contiguous descriptor per partition; split across the three
            # DGE-capable engines
            biasT_sbs = []
            for gi, (h0, nh) in enumerate(((0, 3), (3, 3), (6, 2))):
                bt_g = const.tile([JP, nh * JT * N], f8, name=f"biasTg{gi}")
                eng = (nc.sync, nc.scalar, nc.gpsimd)[gi]
                eng.dma_start(
                    out=bt_g,
                    in_=biasT_ext[:, h0:h0 + nh, :].rearrange("p h tn -> p (h tn)"))
                for hh in range(nh):
                    biasT_sbs.append((bt_g, hh))
            # on-device exp(bias) expansion (bf16, 626-stride blocks for TT
            # alignment): used by the DVE bias path from batch 1 on; runs on
            # ScalarE during the startup DMA window
            ebias_sb = const.tile([JP, H, JT, 626], bf16)
            for h in range(H):
                bt_g, hh = biasT_sbs[h]
                nc.scalar.activation(
                    out=ebias_sb[0:JP, h, :, 0:N],
                    in_=bt_g[0:JP, hh * JT * N:(hh + 1) * JT * N].rearrange(
                        "p (t n) -> p t n", t=JT),
                    func=mybir.ActivationFunctionType.Exp)
            woutp_sb = const.tile([128, 4, D], bf16)
            nc.scalar.dma_start(out=woutp_sb, in_=woutp_ext[:, :, :])  # needed late (proj)
            ident = const.tile([128, 128], bf16)
            make_identity(nc, ident)
            ident8 = const.tile([128, 128], f8)
            nc.vector.tensor_copy(ident8, ident)

            x_tiles = {}

            def load_x(b):
                x_t = xp.tile([128, 2, N], bf16, tag="x")
                nc.sync.dma_start(out=x_t, in_=x_ext[b].rearrange("(k p) n -> p k n", p=128))
                x_tiles[b] = x_t

            # two persistent v buffers in [j, jt, head, 1|dh|0] layout: ones
            # col 0 puts the softmax denominator at av row 0 / 64 (custom
            # reciprocal op only works on base-partition-0 APs); zero cols
            # 33-63 keep av rows finite. Ones/zeros are set once; per batch
            # only cols 1-32 are rewritten.
            v_bufs = []
            for vb in range(2):
                v_t = const.tile([JP, JT, H, 2 * DH], bf16, name=f"vbuf{vb}")
                nc.gpsimd.memset(v_t, 0.0)
                nc.gpsimd.memset(v_t[:, :, :, 0:1], 1.0)
                v_bufs.append(v_t)

            def compute_qkv(b, x_sb):
                # q^T, k^T tiles: qkT_sb[:, m, :], m in 0..3 (q: 0-1, k: 2-3)
                qkT_sb = qkp.tile([128, 4, N], bf16, tag="qkT")
                for m in range(4):
                    ps = simp.tile([128, N], f32, tag="sim")
                    for kt in range(2):
                        for lo, sz in CHUNKS:
                            nc.tensor.matmul(
                                ps[:, lo:lo + sz],
                                wqkv_sb[:, kt, m * 128:(m + 1) * 128],
                                x_sb[:, kt, lo:lo + sz],
                                start=(kt == 0), stop=(kt == 1))
                    nc.vector.tensor_copy(qkT_sb[:, m, :], ps)

                v_sb = v_bufs[b % 2]
                for nt in range(JT):
                    psv = simp.tile([JP, 2 * DH * H], f32, tag="sim")
                    for kt in range(2):
                        nc.tensor.matmul(
                            psv[:, :D],
                            x_sb[:, kt, nt * JP:(nt + 1) * JP],
                            wqkv_sb[:, kt, 2 * D:3 * D],
                            start=(kt == 0), stop=(kt == 1))
                    nc.vector.tensor_copy(
                        v_sb[:, nt, :, 1:DH + 1],
                        psv[:, :D].rearrange("p (h d) -> p h d", h=H))
                return qkT_sb, v_sb

            load_x(0)
            cur_qkv = compute_qkv(0, x_tiles.pop(0))
            for b in range(BL):
                qkT_sb, v_sb = cur_qkv
                if b + 1 < BL:
                    load_x(b + 1)   # prefetch ahead of this batch's output DMAs

                outT_sb = otp.tile([128, 4, N], bf16)
                # iterate (pair, jt); issue av(k-1) after sim/bias/exp(k) so the
                # PE never stalls in-order on es(k) being produced by ACT
                iters = [(g, jt) for g in range(4) for jt in range(JT)]
                av_tiles = {}
                pending = []

                def issue_sim(g, jt, dve_bias):
                    # per-head sim tiles in separate banks: the paired heads'
                    # QK matmuls interleave on distinct PE row groups and run
                    # concurrently (32-row sub-array tiling)
                    sims = [simp.tile([JP, N], f32, tag="sim", name=f"sim{m}")
                            for m in range(2)]
                    for ci in range(2):
                        for m in range(2):
                            h = 2 * g + m
                            hq, mt = (h % 4) * 32, h // 4
                            lo, sz = CHUNKS[ci]
                            nc.tensor.matmul(
                                sims[m][:, lo:lo + sz],
                                qkT_sb[hq:hq + 32, 2 + mt, jt * JP:(jt + 1) * JP],
                                qkT_sb[hq:hq + 32, mt, lo:lo + sz],
                                start=True, stop=(dve_bias),
                                tile_position=(hq, 0))
                    es = esp.tile([JP, 2, 626], bf16, tag="es")
                    if dve_bias:
                        # bias as a post-exp DVE multiply with the on-device
                        # expanded exp(bias) table (bf16 2x mode, 626-stride)
                        es0 = esp.tile([JP, 2, 626], bf16, tag="es0")
                        for m in range(2):
                            h = 2 * g + m
                            nc.scalar.activation(
                                out=es0[:, m, 0:N], in_=sims[m],
                                func=mybir.ActivationFunctionType.Exp)
                            nc.vector.tensor_mul(
                                es[:, m, 0:N], es0[:, m, 0:N],
                                ebias_sb[0:JP, h, jt, 0:N])
                    else:
                        for m in range(2):
                            h = 2 * g + m
                            bt_g, hh = biasT_sbs[h]
                            boff = (hh * JT + jt) * N
                            for lo, sz in CHUNKS:
                                nc.tensor.matmul(
                                    sims[m][:, lo:lo + sz],
                                    ident8[0:JP, 0:JP],
                                    bt_g[0:JP, boff + lo:boff + lo + sz],
                                    start=False, stop=True, tile_position=(0, 0))
                            nc.scalar.activation(
                                out=es[:, m, 0:N], in_=sims[m],
                                func=mybir.ActivationFunctionType.Exp)
                    return es

                def issue_av(g, jt, es):
                    if jt == 0:
                        av_t = avp.tile([128, 2, 512], f32, tag="av")
                        av_tiles[g] = av_t
                    av = av_tiles[g]
                    # two concurrent accumulation groups at disjoint partition
                    # ranges of the same banks: HW has_written clears are
                    # partition-selective (probe-verified); the sim's
                    # zero-region check is conservative, hence skip_group_check
                    for m in range(2):
                        h = 2 * g + m
                        ro = 64 * m
                        for ci, (lo, sz) in enumerate(CHUNKS):
                            nc.tensor.matmul(
                                av[ro:ro + 2 * DH, ci, 0:sz],
                                v_sb[0:JP, jt, h, :],
                                es[0:JP, m, lo:lo + sz],
                                start=(jt == 0), stop=(jt == JT - 1),
                                tile_position=(0, ro), skip_group_check=True)

                def issue_norm(g):
                    av_t = av_tiles.pop(g)
                    # reciprocal_approx_fast and partition_broadcast both
                    # require base-partition-0 APs on HW; denom A is at av
                    # row 0 (ones col 0), denom B at row 64 needs a plain
                    # shift-copy to partition 0 first
                    rbs = []
                    for m in range(2):
                        dr = 64 * m
                        rcpc = rcpp.tile([1, N], f32, tag="rcpc")
                        if m == 0:
                            for ci, (lo, sz) in enumerate(CHUNKS):
                                nc.vector.reciprocal_approx_fast(
                                    rcpc[:, lo:lo + sz], av_t[0:1, ci, 0:sz])
                        else:
                            den = rcpp.tile([1, N], f32, tag="den")
                            for ci, (lo, sz) in enumerate(CHUNKS):
                                nc.vector.tensor_copy(
                                    den[0:1, lo:lo + sz],
                                    av_t[dr:dr + 1, ci, 0:sz])
                            for lo, sz in CHUNKS:
                                nc.vector.reciprocal_approx_fast(
                                    rcpc[:, lo:lo + sz], den[0:1, lo:lo + sz])
                        rb = rbp.tile([128, N], f32, tag="rb")
                        nc.gpsimd.partition_broadcast(rb, rcpc, channels=128)
                        rbs.append(rb)
                    for m in range(2):
                        ro = 64 * m
                        for ci, (lo, sz) in enumerate(CHUNKS):
                            nc.vector.tensor_mul(
                                outT_sb[ro:ro + 64, g, lo:lo + sz],
                                av_t[ro:ro + 64, ci, 0:sz],
                                rbs[m][ro:ro + 64, lo:lo + sz])

                for k, (g, jt) in enumerate(iters + [(None, None)]):
                    if g is not None:
                        # batch 0 keeps the PE bias path while the ebias
                        # expansion finishes behind the startup DMAs
                        es = issue_sim(g, jt, dve_bias=(b > 0 and g >= 1))
                        pending.append((g, jt, es))
                    if len(pending) > (1 if g is not None else 0):
                        pg, pjt, pes = pending.pop(0)
                        issue_av(pg, pjt, pes)
                        if pjt == JT - 1:
                            issue_norm(pg)

                # next batch's qkv runs before this batch's projection so the
                # PE stays busy while the last pair's normalize chain drains
                if b + 1 < BL:
                    cur_qkv = compute_qkv(b + 1, x_tiles.pop(b + 1))

                # output projection with pair-major permuted w_out
                for ct in range(2):
                    psp = simp.tile([128, N], f32, tag="sim")
                    for g in range(4):
                        for lo, sz in CHUNKS:
                            nc.tensor.matmul(
                                psp[:, lo:lo + sz],
                                woutp_sb[:, g, ct * 128:(ct + 1) * 128],
                                outT_sb[:, g, lo:lo + sz],
                                start=(g == 0), stop=(g == 3))
                    o_t = resp.tile([128, N], bf16)
                    nc.vector.tensor_copy(o_t, psp)
                    nc.sync.dma_start(out=out_ext[b, ct * 128:(ct + 1) * 128, :], in_=o_t)

    nc.compile()
    return nc


def _get_nc():
    if "nc" not in _cache:
        _cache["nc"] = _build()
    return _cache["nc"]


def prep_inputs(x, w_qkv, rel_emb, w_out, rel_idx):
    bf = ml_dtypes.bfloat16

    wqkv_s = np.array(w_qkv, dtype=np.float32, copy=True)
    wqkv_s[:, :D] *= SCALE                      # fold q scaling into weights
    wqkv_b = wqkv_s.astype(bf)

    # pair-major permuted w_out matching outT rows: pair g -> rows 1-32 (head
    # 2g) and 65-96 (head 2g+1); rows 0/64 (denominator) and the rest are zero
    wper = np.zeros((4, 128, D), dtype=np.float32)
    wf = np.asarray(w_out, dtype=np.float32)
    for g in range(4):
        wper[g, 1:DH + 1] = wf[(2 * g) * DH:(2 * g + 1) * DH]
        wper[g, 65:65 + DH] = wf[(2 * g + 1) * DH:(2 * g + 2) * DH]
    woutp = np.ascontiguousarray(wper.transpose(1, 0, 2)).astype(bf)

    # bias[h, i, j] = rel_emb[rel_idx[i, j], h];  biasT[h, j, i] = bias[h, i, j]
    # laid out [H, JP, JT*N] so each head is one contiguous-per-partition DMA;
    # fp8e4m3 (values are tiny, |b| < ~0.1) halves the startup DMA bytes
    bias = np.asarray(rel_emb, dtype=np.float32)[np.asarray(rel_idx)]   # [i, j, h]
    biasT = np.ascontiguousarray(
        bias.transpose(2, 1, 0).reshape(H, JT, JP, N).transpose(2, 0, 1, 3)
    ).reshape(JP, H, JT * N).astype(ml_dtypes.float8_e4m3)

    xf = np.asarray(x, dtype=np.float32).reshape(B, D, N).astype(bf)
    return [
        {"x": xf[c * BL:(c + 1) * BL], "wqkv": wqkv_b, "woutp": woutp,
         "biasT": biasT}
        for c in range(NC)
    ]


def kernel(x, w_qkv, rel_emb, w_out, rel_idx):
    from concourse.bass_utils import run_bass_kernel_spmd

    nc = _get_nc()
    in_maps = prep_inputs(x, w_qkv, rel_emb, w_out, rel_idx)
    res = run_bass_kernel_spmd(nc, in_maps, list(range(NC)))
    out = np.concatenate(
        [np.asarray(res.results[c]["out"], dtype=np.float32) for c in range(NC)],
        axis=0)
    return out.reshape(B, D, WS, WS)


# revision 65
# speedup vs baseline: 1.3281x; 1.3281x over previous
"""Trainium2 Bass kernel for windowed attention with relative-position bias.

Problem (hardcoded): x [32, 256, 25, 25] f32, w_qkv [256, 768], rel_emb [2401, 8],
w_out [256, 256], rel_idx [625, 625] int32. 8 heads of dim 32, n = 625 tokens.

Sharding: data-parallel over batch; 4 batches per core on 8 NeuronCores; weights
and bias replicated. No collectives.

Per-core dataflow (bf16 matmuls, f32 PSUM accumulate):
  qkv^T = w_qkv^T @ x          -> qT,kT tiles [32h..., 625]  (q pre-scaled on host)
  v     = x^T @ w_v            -> v tiles [125, 5jt, 8h, 1|dh|0] (ones col 0)
  sim^T = k_h^T q_h (+ bias^T via identity-matmul accumulate, raw bias bf16)
          pair-combined PSUM tiles [125, 2, 625]; paired heads on distinct PE
          row groups (2-way row tiling)
  es    = exp(sim^T)           (one ScalarE activation per pair-jt, -> bf16)
  av^T  = [1|v|0]^T @ es       col strips 0/64 in one shared PSUM tile
                                (partition-disjoint concurrent accum groups);
                                ones row 0/64 gives the softmax denominator
  outT  = av * bcast(1/denom)  (reciprocal_approx_fast at partition 0 + gpsimd
                                broadcast + per-head TT)
  out^T = w_outP^T @ outT      per-pair K blocks of host-permuted w_out -> HBM
"""

import sys

if "/opt/trn_rl_repo" not in sys.path:
    sys.path.insert(0, "/opt/trn_rl_repo")

import numpy as np
import ml_dtypes

B, D, WS = 32, 256, 25
N = WS * WS            # 625
H, DH = 8, 32
NC = 8                 # cores
BL = B // NC           # 4 batches per core
SCALE = DH ** -0.5
JT = 5                 # j tiles of 125
JP = N // JT           # 125
CHUNKS = ((0, 512), (512, 113))             # i chunks for 625-wide psum at offset 0
CHUNKS_M = (((0, 512), (512, 113)),         # bank-aligned chunks for sim[:, m, :]
            ((0, 399), (399, 226)))         # (m=1 starts at f32 offset 625)

_cache = {}


def _build():
    import concourse.bass as bass
    from concourse import bacc, mybir
    from concourse.tile import TileContext
    from concourse.masks import make_identity

    f32 = mybir.dt.float32
    bf16 = mybir.dt.bfloat16
    f8 = mybir.dt.float8e4

    nc = bacc.Bacc()
    x_ext = nc.declare_dram_parameter("x", [BL, D, N], bf16, isOutput=False)
    wqkv_ext = nc.declare_dram_parameter("wqkv", [D, 3 * D], bf16, isOutput=False)
    woutp_ext = nc.declare_dram_parameter("woutp", [128, 4, D], bf16, isOutput=False)
    biasT_ext = nc.declare_dram_parameter("biasT", [H, JP, JT * N], f8, isOutput=False)
    out_ext = nc.declare_dram_parameter("out", [BL, D, N], bf16, isOutput=True)

    with TileContext(nc) as tc:
        with (
            tc.tile_pool(name="const", bufs=1) as const,
            tc.tile_pool(name="xp", bufs=2) as xp,
            tc.tile_pool(name="qk", bufs=2) as qkp,
            tc.tile_pool(name="vp", bufs=2) as vp,
            tc.tile_pool(name="es", bufs=4) as esp,
            tc.tile_pool(name="rcp", bufs=3) as rcpp,
            tc.tile_pool(name="rb", bufs=2) as rbp,
            tc.tile_pool(name="ot", bufs=2) as otp,
            tc.tile_pool(name="res", bufs=2) as resp,
            tc.tile_pool(name="sim", bufs=3, space="PSUM") as simp,
            tc.tile_pool(name="avp", bufs=1, space="PSUM") as avp,
        ):
            wqkv_sb = const.tile([128, 2, 3 * D], bf16)
            nc.sync.dma_start(out=wqkv_sb, in_=wqkv_ext.rearrange("(k p) c -> p k c", p=128))
            biasT_sbs = []
            for h in range(H):
                bt_h = const.tile([JP, JT, N], f8, name=f"biasT{h}")
                eng = (nc.sync, nc.scalar, nc.gpsimd)[h % 3]
                eng.dma_start(
                    out=bt_h, in_=biasT_ext[h].rearrange("p (t n) -> p t n", t=JT))
                biasT_sbs.append(bt_h)
            woutp_sb = const.tile([128, 4, D], bf16)
            nc.scalar.dma_start(out=woutp_sb, in_=woutp_ext[:, :, :])  # needed late (proj)
            ident = const.tile([128, 128], bf16)
            make_identity(nc, ident)
            ident8 = const.tile([128, 128], f8)
            nc.vector.tensor_copy(ident8, ident)

            x_tiles = {}

            def load_x(b):
                x_t = xp.tile([128, 2, N], bf16, tag="x")
                nc.sync.dma_start(out=x_t, in_=x_ext[b].rearrange("(k p) n -> p k n", p=128))
                x_tiles[b] = x_t

            # two persistent v buffers in [j, jt, head, 1|dh|0] layout: ones
            # col 0 puts the softmax denominator at av row 0 / 64 (custom
            # reciprocal op only works on base-partition-0 APs); zero cols
            # 33-63 keep av rows finite. Ones/zeros are set once; per batch
            # only cols 1-32 are rewritten.
            v_bufs = []
            for vb in range(2):
                v_t = const.tile([JP, JT, H, 2 * DH], bf16, name=f"vbuf{vb}")
                nc.gpsimd.memset(v_t, 0.0)
                nc.gpsimd.memset(v_t[:, :, :, 0:1], 1.0)
                v_bufs.append(v_t)

            def compute_qkv(b, x_sb):
                # q^T, k^T tiles: qkT_sb[:, m, :], m in 0..3 (q: 0-1, k: 2-3)
                qkT_sb = qkp.tile([128, 4, N], bf16, tag="qkT")
                for m in range(4):
                    ps = simp.tile([128, N], f32, tag="sim")
                    for kt in range(2):
                        for lo, sz in CHUNKS:
                            nc.tensor.matmul(
                                ps[:, lo:lo + sz],
                                wqkv_sb[:, kt, m * 128:(m + 1) * 128],
                                x_sb[:, kt, lo:lo + sz],
                                start=(kt == 0), stop=(kt == 1))
                    nc.vector.tensor_copy(qkT_sb[:, m, :], ps)

                v_sb = v_bufs[b % 2]
                for nt in range(JT):
                    psv = simp.tile([JP, 2 * DH * H], f32, tag="sim")
                    for kt in range(2):
                        nc.tensor.matmul(
                            psv[:, :D],
                            x_sb[:, kt, nt * JP:(nt + 1) * JP],
                            wqkv_sb[:, kt, 2 * D:3 * D],
                            start=(kt == 0), stop=(kt == 1))
                    nc.vector.tensor_copy(
                        v_sb[:, nt, :, 1:DH + 1],
                        psv[:, :D].rearrange("p (h d) -> p h d", h=H))
                return qkT_sb, v_sb

            load_x(0)
            cur_qkv = compute_qkv(0, x_tiles.pop(0))
            for b in range(BL):
                qkT_sb, v_sb = cur_qkv
                if b + 1 < BL:
                    load_x(b + 1)   # prefetch ahead of this batch's output DMAs

                outT_sb = otp.tile([128, 4, N], bf16)
                # iterate (pair, jt); issue av(k-1) after sim/bias/exp(k) so the
                # PE never stalls in-order on es(k) being produced by ACT
                iters = [(g, jt) for g in range(4) for jt in range(JT)]
                av_tiles = {}
                pending = []

                def issue_sim(g, jt):
                    # per-head sim tiles in separate banks: the paired heads'
                    # QK matmuls interleave on distinct PE row groups and run
                    # concurrently (32-row sub-array tiling)
                    sims = [simp.tile([JP, N], f32, tag="sim", name=f"sim{m}")
                            for m in range(2)]
                    for ci in range(2):
                        for m in range(2):
                            h = 2 * g + m
                            hq, mt = (h % 4) * 32, h // 4
                            lo, sz = CHUNKS[ci]
                            nc.tensor.matmul(
                                sims[m][:, lo:lo + sz],
                                qkT_sb[hq:hq + 32, 2 + mt, jt * JP:(jt + 1) * JP],
                                qkT_sb[hq:hq + 32, mt, lo:lo + sz],
                                start=True, stop=False, tile_position=(hq, 0))
                    es = esp.tile([JP, 2, N], bf16, tag="es")
                    for m in range(2):
                        h = 2 * g + m
                        for lo, sz in CHUNKS:
                            nc.tensor.matmul(
                                sims[m][:, lo:lo + sz],
                                ident8[0:JP, 0:JP],
                                biasT_sbs[h][0:JP, jt, lo:lo + sz],
                                start=False, stop=True, tile_position=(0, 0))
                        nc.scalar.activation(out=es[:, m, :], in_=sims[m],
                                             func=mybir.ActivationFunctionType.Exp)
                    return es

                def issue_av(g, jt, es):
                    if jt == 0:
                        av_t = avp.tile([128, 2, 512], f32, tag="av")
                        av_tiles[g] = av_t
                    av = av_tiles[g]
                    # two concurrent accumulation groups at disjoint partition
                    # ranges of the same banks: HW has_written clears are
                    # partition-selective (probe-verified); the sim's
                    # zero-region check is conservative, hence skip_group_check
                    for m in range(2):
                        h = 2 * g + m
                        ro = 64 * m
                        for ci, (lo, sz) in enumerate(CHUNKS):
                            nc.tensor.matmul(
                                av[ro:ro + 2 * DH, ci, 0:sz],
                                v_sb[0:JP, jt, h, :],
                                es[0:JP, m, lo:lo + sz],
                                start=(jt == 0), stop=(jt == JT - 1),
                                tile_position=(0, ro), skip_group_check=True)

                def issue_norm(g):
                    av_t = av_tiles.pop(g)
                    # reciprocal_approx_fast and partition_broadcast both
                    # require base-partition-0 APs on HW; denom A is at av
                    # row 0 (ones col 0), denom B at row 64 needs a plain
                    # shift-copy to partition 0 first
                    rbs = []
                    for m in range(2):
                        dr = 64 * m
                        rcpc = rcpp.tile([1, N], f32, tag="rcpc")
                        if m == 0:
                            for ci, (lo, sz) in enumerate(CHUNKS):
                                nc.vector.reciprocal_approx_fast(
                                    rcpc[:, lo:lo + sz], av_t[0:1, ci, 0:sz])
                        else:
                            den = rcpp.tile([1, N], f32, tag="den")
                            for ci, (lo, sz) in enumerate(CHUNKS):
                                nc.vector.tensor_copy(
                                    den[0:1, lo:lo + sz],
                                    av_t[dr:dr + 1, ci, 0:sz])
                            for lo, sz in CHUNKS:
                                nc.vector.reciprocal_approx_fast(
                                    rcpc[:, lo:lo + sz], den[0:1, lo:lo + sz])
                        rb = rbp.tile([128, N], f32, tag="rb")
                        nc.gpsimd.partition_broadcast(rb, rcpc, channels=128)
                        rbs.append(rb)
                    for m in range(2):
                        ro = 64 * m
                        for ci, (lo, sz) in enumerate(CHUNKS):
                            nc.vector.tensor_mul(
                                outT_sb[ro:ro + 64, g, lo:lo + sz],
                                av_t[ro:ro + 64, ci, 0:sz],
                                rbs[m][ro:ro + 64, lo:lo + sz])

                for k, (g, jt) in enumerate(iters + [(None, None)]):
                    if g is not None:
                        es = issue_sim(g, jt)
                        pending.append((g, jt, es))
                    if len(pending) > (1 if g is not None else 0):
                        pg, pjt, pes = pending.pop(0)
                        issue_av(pg, pjt, pes)
                        if pjt == JT - 1:
                            issue_norm(pg)

                # next batch's qkv runs before this batch's projection so the
                # PE stays busy while the last pair's normalize chain drains
                if b + 1 < BL:
                    cur_qkv = compute_qkv(b + 1, x_tiles.pop(b + 1))

                # output projection with pair-major permuted w_out
                for ct in range(2):
                    psp = simp.tile([128, N], f32, tag="sim")
                    for g in range(4):
                        for lo, sz in CHUNKS:
                            nc.tensor.matmul(
                                psp[:, lo:lo + sz],
                                woutp_sb[:, g, ct * 128:(ct + 1) * 128],
                                outT_sb[:, g, lo:lo + sz],
                                start=(g == 0), stop=(g == 3))
                    o_t = resp.tile([128, N], bf16)
                    nc.vector.tensor_copy(o_t, psp)
                    nc.sync.dma_start(out=out_ext[b, ct * 128:(ct + 1) * 128, :], in_=o_t)

    nc.compile()
    return nc


def _get_nc():
    if "nc" not in _cache:
        _cache["nc"] = _build()
    return _cache["nc"]


def prep_inputs(x, w_qkv, rel_emb, w_out, rel_idx):
    bf = ml_dtypes.bfloat16

    wqkv_s = np.array(w_qkv, dtype=np.float32, copy=True)
    wqkv_s[:, :D] *= SCALE                      # fold q scaling into weights
    wqkv_b = wqkv_s.astype(bf)

    # pair-major permuted w_out matching outT rows: pair g -> rows 1-32 (head
    # 2g) and 65-96 (head 2g+1); rows 0/64 (denominator) and the rest are zero
    wper = np.zeros((4, 128, D), dtype=np.float32)
    wf = np.asarray(w_out, dtype=np.float32)
    for g in range(4):
        wper[g, 1:DH + 1] = wf[(2 * g) * DH:(2 * g + 1) * DH]
        wper[g, 65:65 + DH] = wf[(2 * g + 1) * DH:(2 * g + 2) * DH]
    woutp = np.ascontiguousarray(wper.transpose(1, 0, 2)).astype(bf)

    # bias[h, i, j] = rel_emb[rel_idx[i, j], h];  biasT[h, j, i] = bias[h, i, j]
    # laid out [H, JP, JT*N] so each head is one contiguous-per-partition DMA;
    # fp8e4m3 (values are tiny, |b| < ~0.1) halves the startup DMA bytes
    bias = np.asarray(rel_emb, dtype=np.float32)[np.asarray(rel_idx)]   # [i, j, h]
    biasT = np.ascontiguousarray(
        bias.transpose(2, 1, 0).reshape(H, JT, JP, N).transpose(0, 2, 1, 3)
    ).reshape(H, JP, JT * N).astype(ml_dtypes.float8_e4m3)

    xf = np.asarray(x, dtype=np.float32).reshape(B, D, N).astype(bf)
    return [
        {"x": xf[c * BL:(c + 1) * BL], "wqkv": wqkv_b, "woutp": woutp,
         "biasT": biasT}
        for c in range(NC)
    ]


def kernel(x, w_qkv, rel_emb, w_out, rel_idx):
    from concourse.bass_utils import run_bass_kernel_spmd

    nc = _get_nc()
    in_maps = prep_inputs(x, w_qkv, rel_emb, w_out, rel_idx)
    res = run_bass_kernel_spmd(nc, in_maps, list(range(NC)))
    out = np.concatenate(
        [np.asarray(res.results[c]["out"], dtype=np.float32) for c in range(NC)],
        axis=0)
    return out.reshape(B, D, WS, WS)


# revision 66
# speedup vs baseline: 1.3381x; 1.0075x over previous
"""Trainium2 Bass kernel for windowed attention with relative-position bias.

Problem (hardcoded): x [32, 256, 25, 25] f32, w_qkv [256, 768], rel_emb [2401, 8],
w_out [256, 256], rel_idx [625, 625] int32. 8 heads of dim 32, n = 625 tokens.

Sharding: data-parallel over batch; 4 batches per core on 8 NeuronCores; weights
and bias replicated. No collectives.

Per-core dataflow (bf16 matmuls, f32 PSUM accumulate):
  qkv^T = w_qkv^T @ x          -> qT,kT tiles [32h..., 625]  (q pre-scaled on host)
  v     = x^T @ w_v            -> v tiles [125, 5jt, 8h, 1|dh|0] (ones col 0)
  sim^T = k_h^T q_h (+ bias^T via identity-matmul accumulate, raw bias bf16)
          pair-combined PSUM tiles [125, 2, 625]; paired heads on distinct PE
          row groups (2-way row tiling)
  es    = exp(sim^T)           (one ScalarE activation per pair-jt, -> bf16)
  av^T  = [1|v|0]^T @ es       col strips 0/64 in one shared PSUM tile
                                (partition-disjoint concurrent accum groups);
                                ones row 0/64 gives the softmax denominator
  outT  = av * bcast(1/denom)  (reciprocal_approx_fast at partition 0 + gpsimd
                                broadcast + per-head TT)
  out^T = w_outP^T @ outT      per-pair K blocks of host-permuted w_out -> HBM
"""

import sys

if "/opt/trn_rl_repo" not in sys.path:
    sys.path.insert(0, "/opt/trn_rl_repo")

import numpy as np
import ml_dtypes

B, D, WS = 32, 256, 25
N = WS * WS            # 625
H, DH = 8, 32
NC = 8                 # cores
BL = B // NC           # 4 batches per core
SCALE = DH ** -0.5
JT = 5                 # j tiles of 125
JP = N // JT           # 125
CHUNKS = ((0, 512), (512, 113))             # i chunks for 625-wide psum at offset 0
CHUNKS_M = (((0, 512), (512, 113)),         # bank-aligned chunks for sim[:, m, :]
            ((0, 399), (399, 226)))         # (m=1 starts at f32 offset 625)

_cache = {}


def _build():
    import concourse.bass as bass
    from concourse import bacc, mybir
    from concourse.tile import TileContext
    from concourse.masks import make_identity

    f32 = mybir.dt.float32
    bf16 = mybir.dt.bfloat16
    f8 = mybir.dt.float8e4

    nc = bacc.Bacc()
    x_ext = nc.declare_dram_parameter("x", [BL, D, N], bf16, isOutput=False)
    wqkv_ext = nc.declare_dram_parameter("wqkv", [D, 3 * D], bf16, isOutput=False)
    woutp_ext = nc.declare_dram_parameter("woutp", [128, 4, D], bf16, isOutput=False)
    biasT_ext = nc.declare_dram_parameter("biasT", [H, JP, JT * N], f8, isOutput=False)
    out_ext = nc.declare_dram_parameter("out", [BL, D, N], bf16, isOutput=True)

    with TileContext(nc) as tc:
        with (
            tc.tile_pool(name="const", bufs=1) as const,
            tc.tile_pool(name="xp", bufs=2) as xp,
            tc.tile_pool(name="qk", bufs=2) as qkp,
            tc.tile_pool(name="vp", bufs=2) as vp,
            tc.tile_pool(name="es", bufs=6) as esp,
            tc.tile_pool(name="rcp", bufs=4) as rcpp,
            tc.tile_pool(name="rb", bufs=3) as rbp,
            tc.tile_pool(name="ot", bufs=2) as otp,
            tc.tile_pool(name="res", bufs=2) as resp,
            tc.tile_pool(name="sim", bufs=3, space="PSUM") as simp,
            tc.tile_pool(name="avp", bufs=1, space="PSUM") as avp,
        ):
            wqkv_sb = const.tile([128, 2, 3 * D], bf16)
            nc.sync.dma_start(out=wqkv_sb, in_=wqkv_ext.rearrange("(k p) c -> p k c", p=128))
            biasT_sbs = []
            for h in range(H):
                bt_h = const.tile([JP, JT, N], f8, name=f"biasT{h}")
                eng = (nc.sync, nc.scalar, nc.gpsimd)[h % 3]
                eng.dma_start(
                    out=bt_h, in_=biasT_ext[h].rearrange("p (t n) -> p t n", t=JT))
                biasT_sbs.append(bt_h)
            woutp_sb = const.tile([128, 4, D], bf16)
            nc.scalar.dma_start(out=woutp_sb, in_=woutp_ext[:, :, :])  # needed late (proj)
            ident = const.tile([128, 128], bf16)
            make_identity(nc, ident)
            ident8 = const.tile([128, 128], f8)
            nc.vector.tensor_copy(ident8, ident)

            x_tiles = {}

            def load_x(b):
                x_t = xp.tile([128, 2, N], bf16, tag="x")
                nc.sync.dma_start(out=x_t, in_=x_ext[b].rearrange("(k p) n -> p k n", p=128))
                x_tiles[b] = x_t

            # two persistent v buffers in [j, jt, head, 1|dh|0] layout: ones
            # col 0 puts the softmax denominator at av row 0 / 64 (custom
            # reciprocal op only works on base-partition-0 APs); zero cols
            # 33-63 keep av rows finite. Ones/zeros are set once; per batch
            # only cols 1-32 are rewritten.
            v_bufs = []
            for vb in range(2):
                v_t = const.tile([JP, JT, H, 2 * DH], bf16, name=f"vbuf{vb}")
                nc.gpsimd.memset(v_t, 0.0)
                nc.gpsimd.memset(v_t[:, :, :, 0:1], 1.0)
                v_bufs.append(v_t)

            def compute_qkv(b, x_sb):
                # q^T, k^T tiles: qkT_sb[:, m, :], m in 0..3 (q: 0-1, k: 2-3)
                qkT_sb = qkp.tile([128, 4, N], bf16, tag="qkT")
                for m in range(4):
                    ps = simp.tile([128, N], f32, tag="sim")
                    for kt in range(2):
                        for lo, sz in CHUNKS:
                            nc.tensor.matmul(
                                ps[:, lo:lo + sz],
                                wqkv_sb[:, kt, m * 128:(m + 1) * 128],
                                x_sb[:, kt, lo:lo + sz],
                                start=(kt == 0), stop=(kt == 1))
                    nc.vector.tensor_copy(qkT_sb[:, m, :], ps)

                v_sb = v_bufs[b % 2]
                for nt in range(JT):
                    psv = simp.tile([JP, 2 * DH * H], f32, tag="sim")
                    for kt in range(2):
                        nc.tensor.matmul(
                            psv[:, :D],
                            x_sb[:, kt, nt * JP:(nt + 1) * JP],
                            wqkv_sb[:, kt, 2 * D:3 * D],
                            start=(kt == 0), stop=(kt == 1))
                    nc.vector.tensor_copy(
                        v_sb[:, nt, :, 1:DH + 1],
                        psv[:, :D].rearrange("p (h d) -> p h d", h=H))
                return qkT_sb, v_sb

            load_x(0)
            cur_qkv = compute_qkv(0, x_tiles.pop(0))
            for b in range(BL):
                qkT_sb, v_sb = cur_qkv
                if b + 1 < BL:
                    load_x(b + 1)   # prefetch ahead of this batch's output DMAs

                outT_sb = otp.tile([128, 4, N], bf16)
                # iterate (pair, jt); issue av(k-1) after sim/bias/exp(k) so the
                # PE never stalls in-order on es(k) being produced by ACT
                iters = [(g, jt) for g in range(4) for jt in range(JT)]
                av_tiles = {}
                pending = []

                def issue_sim(g, jt):
                    # per-head sim tiles in separate banks: the paired heads'
                    # QK matmuls interleave on distinct PE row groups and run
                    # concurrently (32-row sub-array tiling)
                    sims = [simp.tile([JP, N], f32, tag="sim", name=f"sim{m}")
                            for m in range(2)]
                    for ci in range(2):
                        for m in range(2):
                            h = 2 * g + m
                            hq, mt = (h % 4) * 32, h // 4
                            lo, sz = CHUNKS[ci]
                            nc.tensor.matmul(
                                sims[m][:, lo:lo + sz],
                                qkT_sb[hq:hq + 32, 2 + mt, jt * JP:(jt + 1) * JP],
                                qkT_sb[hq:hq + 32, mt, lo:lo + sz],
                                start=True, stop=False, tile_position=(hq, 0))
                    es = esp.tile([JP, 2, N], bf16, tag="es")
                    for m in range(2):
                        h = 2 * g + m
                        for lo, sz in CHUNKS:
                            nc.tensor.matmul(
                                sims[m][:, lo:lo + sz],
                                ident8[0:JP, 0:JP],
                                biasT_sbs[h][0:JP, jt, lo:lo + sz],
                                start=False, stop=True, tile_position=(0, 0))
                        nc.scalar.activation(out=es[:, m, :], in_=sims[m],
                                             func=mybir.ActivationFunctionType.Exp)
                    return es

                def issue_av(g, jt, es):
                    if jt == 0:
                        av_t = avp.tile([128, 2, 512], f32, tag="av")
                        av_tiles[g] = av_t
                    av = av_tiles[g]
                    # two concurrent accumulation groups at disjoint partition
                    # ranges of the same banks: HW has_written clears are
                    # partition-selective (probe-verified); the sim's
                    # zero-region check is conservative, hence skip_group_check
                    for m in range(2):
                        h = 2 * g + m
                        ro = 64 * m
                        for ci, (lo, sz) in enumerate(CHUNKS):
                            nc.tensor.matmul(
                                av[ro:ro + 2 * DH, ci, 0:sz],
                                v_sb[0:JP, jt, h, :],
                                es[0:JP, m, lo:lo + sz],
                                start=(jt == 0), stop=(jt == JT - 1),
                                tile_position=(0, ro), skip_group_check=True)

                def issue_norm(g):
                    av_t = av_tiles.pop(g)
                    # reciprocal_approx_fast and partition_broadcast both
                    # require base-partition-0 APs on HW; denom A is at av
                    # row 0 (ones col 0), denom B at row 64 needs a plain
                    # shift-copy to partition 0 first
                    rbs = []
                    for m in range(2):
                        dr = 64 * m
                        rcpc = rcpp.tile([1, N], f32, tag="rcpc")
                        if m == 0:
                            for ci, (lo, sz) in enumerate(CHUNKS):
                                nc.vector.reciprocal_approx_fast(
                                    rcpc[:, lo:lo + sz], av_t[0:1, ci, 0:sz])
                        else:
                            den = rcpp.tile([1, N], f32, tag="den")
                            for ci, (lo, sz) in enumerate(CHUNKS):
                                nc.vector.tensor_copy(
                                    den[0:1, lo:lo + sz],
                                    av_t[dr:dr + 1, ci, 0:sz])
                            for lo, sz in CHUNKS:
                                nc.vector.reciprocal_approx_fast(
                                    rcpc[:, lo:lo + sz], den[0:1, lo:lo + sz])
                        rb = rbp.tile([128, N], f32, tag="rb")
                        nc.gpsimd.partition_broadcast(rb, rcpc, channels=128)
                        rbs.append(rb)
                    for m in range(2):
                        ro = 64 * m
                        for ci, (lo, sz) in enumerate(CHUNKS):
                            nc.vector.tensor_mul(
                                outT_sb[ro:ro + 64, g, lo:lo + sz],
                                av_t[ro:ro + 64, ci, 0:sz],
                                rbs[m][ro:ro + 64, lo:lo + sz])

                for k, (g, jt) in enumerate(iters + [(None, None)]):
                    if g is not None:
                        es = issue_sim(g, jt)
                        pending.append((g, jt, es))
                    if len(pending) > (1 if g is not None else 0):
                        pg, pjt, pes = pending.pop(0)
                        issue_av(pg, pjt, pes)
                        if pjt == JT - 1:
                            issue_norm(pg)

                # next batch's qkv runs before this batch's projection so the
                # PE stays busy while the last pair's normalize chain drains
                if b + 1 < BL:
                    cur_qkv = compute_qkv(b + 1, x_tiles.pop(b + 1))

                # output projection with pair-major permuted w_out
                for ct in range(2):
                    psp = simp.tile([128, N], f32, tag="sim")
                    for g in range(4):
                        for lo, sz in CHUNKS:
                            nc.tensor.matmul(
                                psp[:, lo:lo + sz],
                                woutp_sb[:, g, ct * 128:(ct + 1) * 128],
                                outT_sb[:, g, lo:lo + sz],
                                start=(g == 0), stop=(g == 3))
                    o_t = resp.tile([128, N], bf16)
                    nc.vector.tensor_copy(o_t, psp)
                    nc.sync.dma_start(out=out_ext[b, ct * 128:(ct + 1) * 128, :], in_=o_t)

    nc.compile()
    return nc


def _get_nc():
    if "nc" not in _cache:
        _cache["nc"] = _build()
    return _cache["nc"]


def prep_inputs(x, w_qkv, rel_emb, w_out, rel_idx):
    bf = ml_dtypes.bfloat16

    wqkv_s = np.array(w_qkv, dtype=np.float32, copy=True)
    wqkv_s[:, :D] *= SCALE                      # fold q scaling into weights
    wqkv_b = wqkv_s.astype(bf)

    # pair-major permuted w_out matching outT rows: pair g -> rows 1-32 (head
    # 2g) and 65-96 (head 2g+1); rows 0/64 (denominator) and the rest are zero
    wper = np.zeros((4, 128, D), dtype=np.float32)
    wf = np.asarray(w_out, dtype=np.float32)
    for g in range(4):
        wper[g, 1:DH + 1] = wf[(2 * g) * DH:(2 * g + 1) * DH]
        wper[g, 65:65 + DH] = wf[(2 * g + 1) * DH:(2 * g + 2) * DH]
    woutp = np.ascontiguousarray(wper.transpose(1, 0, 2)).astype(bf)

    # bias[h, i, j] = rel_emb[rel_idx[i, j], h];  biasT[h, j, i] = bias[h, i, j]
    # laid out [H, JP, JT*N] so each head is one contiguous-per-partition DMA;
    # fp8e4m3 (values are tiny, |b| < ~0.1) halves the startup DMA bytes
    bias = np.asarray(rel_emb, dtype=np.float32)[np.asarray(rel_idx)]   # [i, j, h]
    biasT = np.ascontiguousarray(
        bias.transpose(2, 1, 0).reshape(H, JT, JP, N).transpose(0, 2, 1, 3)
    ).reshape(H, JP, JT * N).astype(ml_dtypes.float8_e4m3)

    xf = np.asarray(x, dtype=np.float32).reshape(B, D, N).astype(bf)
    return [
        {"x": xf[c * BL:(c + 1) * BL], "wqkv": wqkv_b, "woutp": woutp,
         "biasT": biasT}
        for c in range(NC)
    ]


def kernel(x, w_qkv, rel_emb, w_out, rel_idx):
    from concourse.bass_utils import run_bass_kernel_spmd

    nc = _get_nc()
    in_maps = prep_inputs(x, w_qkv, rel_emb, w_out, rel_idx)
    res = run_bass_kernel_spmd(nc, in_maps, list(range(NC)))
    out = np.concatenate(
        [np.asarray(res.results[c]["out"], dtype=np.float32) for c in range(NC)],
        axis=0)
    return out.reshape(B, D, WS, WS)
